# revision 21
# baseline (speedup 1.0000x reference)
"""Action-separated MLP (MoE routing) Trainium2 kernel.

Reference computes all 16 per-action MLPs for every row, then gathers the
selected action's output.  Only the selected expert's output survives, so we
route instead: sort rows by action on the host, run each expert's
512->512->512->1 MLP only on its own rows, and scatter back.  16x fewer FLOPs
than the dense reference.

Distribution: 16 experts over 8 cores, 2 experts per core.  Slot 0 holds the
8 most-loaded experts, slot 1 the 8 least-loaded, each slot padded to its own
capacity, so the shared SPMD NEFF pads to cap0+cap1 columns instead of
2*max_count.

Layout: activations are kept transposed (features on SBUF partitions, rows on
the free dim), which makes every layer a plain lhsT.T @ rhs chain with the
per-partition bias + ReLU fused into one ScalarEngine activation op.  The
DRAM image of each expert's xt is tile-contiguous per partition (a tile's 4
contraction chunks adjacent), so each DMA moves 128 x 2.3-3.5KB contiguous
runs -- descriptor-efficient.  b1/b2 biases and the tiny W3 column ride in
BPAD spare columns at the head of the tile-0 block (dedicated small DMAs are
descriptor-overhead-bound).

Matmuls run in bf16 (inputs quantized on host): 1 cycle/row streaming and
LDWEIGHTS gets the compiler-automatic Fast Weight Load (disabled for fp32
dtypes), so weight loads hide behind the matmul stream.  Measured rel err vs
the fp32 reference ~5e-3, inside the 2e-2 gate.

Perf notes (from NTFF traces): the PE is stream-bound; the measured exec
window starts at the framework's const-AP memsets (~6.3us in), so the
preamble before that is free, but walrus's end-of-NEFF re-execution epilogue
(one semaphore clear per sem, split across engines) is fully paid.  The BIR
patch compacts semaphore IDs and caps --max-sem-num to shrink it.
"""

import json
import sys

import numpy as np

sys.path.insert(0, "/opt/trn_rl_repo")

import ml_dtypes  # noqa: E402

import concourse.bass as bass  # noqa: E402
import concourse.mybir as mybir  # noqa: E402
import concourse.tile as tile  # noqa: E402
from concourse.vector_clock import ScopedClock  # noqa: E402

A, D, H = 16, 512, 512
NCORES = 8
EPC = 2  # experts per core
P = 128
KD = D // P  # 4 contraction chunks for layer 1
KH = H // P  # 4 contraction chunks for layers 2/3
N_WARM = 32  # dummy matmuls bridging the gap until the first xt piece lands

# "bf16": cast inputs to bf16 on host, 1 cycle/row matmuls, least DMA, FWL.
# "f32r": fp32 storage, float32r matmuls (1 cycle/row at N>=256, no FWL).
DT_MODE = "bf16"

# spare columns at the head of the tile-0 block, per d-chunk; d=0's head
# carries b1 (cols 0:4), b2 (4:8) and the W3 column (8:12)
BPAD = 32 if DT_MODE == "bf16" else 16

# --max-sem-num handed to walrus for the current compile (set per-BIR by
# _patch_bir below; None = leave walrus at its default)
_MAX_SEM = [None]


def _patch_bir(ant_bir_str):
    """Two BIR rewrites before walrus:

    1. This walrus build rejects >1 embedded sync-wait per instruction.
       Move extra waits onto standalone EventSemaphore ops just before the
       owning instruction (same engine, so program order is preserved).
    2. Compact semaphore IDs to a dense range starting at 3, and cap
       --max-sem-num, to shrink walrus's end-of-NEFF per-semaphore clear
       epilogue (it is inside the measured exec window).
    """
    bir = json.loads(
        ant_bir_str.decode() if isinstance(ant_bir_str, bytes) else ant_bir_str
    )
    used = set()
    for fn in bir.get("functions", []):
        for bb in fn.get("blocks", []):
            for inst in bb.get("instructions", []):
                si = inst.get("sync_info") or {}
                for k in ("on_update", "on_wait"):
                    for u in si.get(k) or []:
                        if isinstance(u, dict) and u.get("sync_type") == "semaphore":
                            used.add(u["id"])
    remap = {old: 3 + i for i, old in enumerate(sorted(used))}
    _MAX_SEM[0] = 3 + len(used) + 8  # slack for walrus-internal sems
    names = bir.get("ant_sem_names")
    if isinstance(names, dict):
        bir["ant_sem_names"] = {
            str(remap.get(int(k), int(k))): v for k, v in names.items()
        }

    for fn in bir.get("functions", []):
        for bb in fn.get("blocks", []):
            new_insts = []
            for inst in bb.get("instructions", []):
                si = inst.get("sync_info") or {}
                for k in ("on_update", "on_wait"):
                    for u in si.get(k) or []:
                        if isinstance(u, dict) and u.get("sync_type") == "semaphore":
                            u["id"] = remap[u["id"]]
                waits = si.get("on_wait") or []
                if len(waits) > 1:
                    for j, w in enumerate(waits[:-1]):
                        new_insts.append(
                            {
                                "debug": inst.get("debug", 0),
                                "engine": inst["engine"],
                                "ins": [],
                                "name": f"{inst['name']}_xw{j}",
                                "opcode": "EventSemaphore",
                                "outs": [],
                                "sync_info": {"on_update": [], "on_wait": [w]},
                            }
                        )
                    si["on_wait"] = [waits[-1]]
                new_insts.append(inst)
            bb["instructions"] = new_insts
    return json.dumps(bir).encode()


def _install_bir_patch():
    from concourse import bass2jax, bass_utils

    if getattr(bass2jax.compile_bir_kernel, "_multiwait_patched", False):
        return

    orig = bass_utils.compile_bir_kernel

    def patched(ant_bir_str, tmpdir, neff_name="file.neff", **kw):
        return orig(_patch_bir(ant_bir_str), tmpdir, neff_name=neff_name, **kw)

    patched._multiwait_patched = True
    bass2jax.compile_bir_kernel = patched

    orig_gwa = bass_utils.get_walrus_args

    def gwa(*a, **kw):
        args = orig_gwa(*a, **kw)
        if _MAX_SEM[0] is not None:
            args = list(args) + [f"--max-sem-num={_MAX_SEM[0]}"]
        return args

    bass_utils.get_walrus_args = gwa


def _dtypes():
    if DT_MODE == "bf16":
        return mybir.dt.bfloat16, ml_dtypes.bfloat16
    if DT_MODE == "f32r":
        return mybir.dt.float32r, np.float32
    return mybir.dt.float32, np.float32


def _xt_geom(cts):
    """Flat per-expert xt layout: tile-0 block [KD, BPAD+ct0], then
    [KD, ct_t] blocks.  Returns (total_width, per-tile col base, per-tile
    d-chunk width)."""
    boffs, dws = [], []
    off = 0
    for t, ct in enumerate(cts):
        w = (BPAD + ct) if t == 0 else ct
        boffs.append(off)
        dws.append(w)
        off += KD * w
    return off, boffs, dws


def build_nc(caps, ctss):
    """Per-core Bass program: slot e handles capacity caps[e] = sum(ctss[e])."""
    io_dt, _ = _dtypes()
    f32 = mybir.dt.float32
    maxct = max(max(cts) for cts in ctss)
    geom = [_xt_geom(cts) for cts in ctss]

    nc = bass.Bass()
    xt_d = [
        nc.dram_tensor(f"xt{e}", [P, geom[e][0]], io_dt, kind="ExternalInput")
        for e in range(EPC)
    ]
    # weights are h-major per partition so the first consumed h-blocks are
    # contiguous leading pieces
    w1_d = nc.dram_tensor("w1", [EPC, P, KH, KD, P], io_dt, kind="ExternalInput")
    w2_d = nc.dram_tensor("w2", [EPC, P, KH, KH, P], io_dt, kind="ExternalInput")
    y_d = [
        nc.dram_tensor(f"y{e}", [1, caps[e]], f32, kind="ExternalOutput")
        for e in range(EPC)
    ]

    RELU = mybir.ActivationFunctionType.Relu

    def _light_drain(self, tick_clock, wait_clock):
        # Single-TileContext program: drain + one barrier is enough.  The
        # stock version adds a second all-engine barrier + semaphore clears
        # (for sem reuse by later contexts) costing ~1.3us of kernel tail;
        # re-execution of the same NEFF was verified safe without them.
        drain_inst = self.nc.sync.drain()
        wait_clock.add_sem_waits(
            drain_inst.ins, ScopedClock({None: tick_clock.global_clock})
        )
        self.nc.all_engine_barrier()
        popped = self.nc._tile_sem_poison_stack.pop()
        assert popped is self._sem_poison

    with tile.TileContext(nc) as tc:
        tc._drain_and_barrier = _light_drain.__get__(tc)
        with (
            tc.tile_pool(name="const", bufs=1) as const,
            tc.tile_pool(name="xt", bufs=2) as xtp,
            tc.tile_pool(name="wts", bufs=2) as wtp,
            tc.tile_pool(name="act", bufs=3) as actp,
            tc.tile_pool(name="out", bufs=4) as outp,
            tc.tile_pool(name="ps", bufs=5, space="PSUM") as psp,
            tc.tile_pool(name="ps3", bufs=2, space="PSUM") as ps3p,
            tc.tile_pool(name="pswarm", bufs=1, space="PSUM") as pswarmp,
        ):
            # PE warm-up: dummy matmuls bridge the PE from program start
            # until the first xt piece lands, keeping the HAM clock gate
            # (which watches PE *array* activity) un-throttled.
            warm_sb = const.tile([P, 64], io_dt, tag="warm")
            ones_sb = const.tile([P, 1], io_dt, tag="ones")
            if io_dt == mybir.dt.bfloat16:
                nc.vector.memset(warm_sb, 0.0)
                nc.vector.memset(ones_sb, 1.0)
            else:
                nc.vector.memset(warm_sb.bitcast(f32), 0.0)
                nc.vector.memset(ones_sb.bitcast(f32), 1.0)
            warm_ps = pswarmp.tile([64, 64], f32, tag="warm_ps")
            for _ in range(N_WARM):
                nc.tensor.matmul(warm_ps[:], warm_sb[:], warm_sb[:],
                                 start=True, stop=True)

            # Two trigger engines = two SW DMA queues, each spreading packets
            # over the 16 HW DMA engines; each trigger costs ~650ns on its
            # issuing engine, so few big tile-contiguous transfers beat many
            # small ones.  Pieces are ordered so each lands just before its
            # first consumer (queues complete in order).
            xt_sb = {}
            w_sb = {}
            for e in range(EPC):
                xt_sb[e] = xtp.tile(
                    [P, geom[e][0]], io_dt, tag=f"xt{e}", name=f"xt_sb{e}"
                )
                w1_sb = wtp.tile([P, KH, KD, P], io_dt, tag="w1", name=f"w1_sb{e}")
                w2_sb = wtp.tile([P, KH, KH, P], io_dt, tag="w2", name=f"w2_sb{e}")
                w_sb[e] = (w1_sb, w2_sb)

            def xt_piece(e, t, frac=(0, 1)):
                # piece frac[0] of frac[1] equal column ranges of tile t
                _, boffs, dws = geom[e]
                w = KD * dws[t]
                lo = boffs[t] + frac[0] * w // frac[1]
                hi = boffs[t] + (frac[0] + 1) * w // frac[1]
                return (xt_sb[e][:, lo:hi], xt_d[e][:, lo:hi])

            def w_piece(which, e, hlo, hhi):
                sb = w_sb[e][which]
                dr = (w1_d, w2_d)[which]
                return (sb[:, hlo:hhi, :, :], dr[e, :, hlo:hhi, :, :])

            # The DMA queues deliver only ~70GB/s each for the first few us,
            # so the pieces gating the first matmul groups are small and
            # spread across all three trigger queues, ordered by deadline.
            sched = [
                (nc.sync,   xt_piece(0, 0, (0, 2))),
                (nc.scalar, w_piece(0, 0, 0, 1)),
                (nc.gpsimd, xt_piece(0, 0, (1, 2))),
                (nc.sync,   w_piece(0, 0, 1, 2)),
                (nc.scalar, w_piece(0, 0, 2, 3)),
                (nc.gpsimd, w_piece(0, 0, 3, 4)),
                (nc.sync,   w_piece(1, 0, 0, 2)),
                (nc.scalar, w_piece(1, 0, 2, 4)),
                (nc.gpsimd, xt_piece(0, 1)),
                (nc.sync,   xt_piece(0, 2)),
                (nc.gpsimd, w_piece(0, 1, 0, 4)),
                (nc.sync,   w_piece(1, 1, 0, 4)),
                (nc.gpsimd, xt_piece(1, 1)),
                (nc.sync,   xt_piece(1, 2)),
                (nc.gpsimd, xt_piece(1, 0)),
            ]
            for eng, (dst, src) in sched:
                eng.dma_start(dst, src)

            for e in range(EPC):
                cts = ctss[e]
                _, boffs, dws = geom[e]
                nt = len(cts)
                w1_sb, w2_sb = w_sb[e]
                bias = xt_sb[e]
                if io_dt != mybir.dt.bfloat16:
                    bias = bias.bitcast(f32)
                    w3f = bias
                else:
                    # tensor_scalar wants a float32 scalar operand; make a
                    # tiny f32 copy of the packed W3 column once per expert
                    w3f = actp.tile([P, KH], f32, tag="w3f", name="w3f")
                    nc.scalar.activation(
                        w3f[:, :], xt_sb[e][:, 2 * KH : 3 * KH],
                        mybir.ActivationFunctionType.Copy,
                    )

                def w3_col(d):
                    if io_dt == mybir.dt.bfloat16:
                        return w3f[:, d : d + 1]
                    return bias[:, 2 * KH + d : 2 * KH + d + 1]

                def rhs(t, d, ct):
                    lo = boffs[t] + d * dws[t] + (BPAD if t == 0 else 0)
                    return xt_sb[e][:, lo : lo + ct]

                t_order = range(nt) if e == 0 else [*range(1, nt), 0]
                for t in t_order:
                    ct = cts[t]
                    h1_sb = actp.tile([P, KH, maxct], io_dt, tag="h1", name="h1_sb")[:, :, :ct]
                    for h in range(KH):
                        ps = psp.tile([P, maxct], f32, tag="ps", name="ps")[:, :ct]
                        for d in range(KD):
                            nc.tensor.matmul(
                                ps[:],
                                w1_sb[:, h, d, :],
                                rhs(t, d, ct),
                                start=(d == 0),
                                stop=(d == KD - 1),
                            )
                        nc.scalar.activation(
                            h1_sb[:, h, :], ps[:], RELU,
                            bias=bias[:, h : h + 1],
                        )
                    h2_sb = actp.tile([P, KH, maxct], io_dt, tag="h2", name="h2_sb")[:, :, :ct]
                    for h in range(KH):
                        ps = psp.tile([P, maxct], f32, tag="ps", name="ps")[:, :ct]
                        for d in range(KH):
                            nc.tensor.matmul(
                                ps[:],
                                w2_sb[:, h, d, :],
                                h1_sb[:, d, :],
                                start=(d == 0),
                                stop=(d == KH - 1),
                            )
                        nc.scalar.activation(
                            h2_sb[:, h, :], ps[:], RELU,
                            bias=bias[:, KH + h : KH + h + 1],
                        )
                    ps3 = ps3p.tile([1, maxct], f32, tag="ps3", name="ps3")[:, :ct]
                    if e == EPC - 1 and t == t_order[-1]:
                        # final tile: classic 4-matmul reduction -- the PE is
                        # idle here and the DVE chain would be exposed tail
                        for d in range(KH):
                            nc.tensor.matmul(
                                ps3[:],
                                xt_sb[e][:, 2 * KH + d : 2 * KH + d + 1],
                                h2_sb[:, d, :],
                                start=(d == 0),
                                stop=(d == KH - 1),
                            )
                    else:
                        # bulk tiles: h2*w3 per chunk + chunk sum on the DVE,
                        # then ONE ones-vector matmul does the 128-partition
                        # reduction -- 1 PE row-stream instead of 4.  The sum
                        # bounces through ACT into a dedicated tensor (for
                        # f32r the BIR verifier accepts only
                        # Activation/DMA/Memset writers for matmul inputs;
                        # for bf16 it also casts f32 -> bf16).
                        hwf = actp.tile([P, KH, maxct], f32, tag="hwf", name="hwf")
                        hws = actp.tile([P, maxct], io_dt, tag="hws", name="hws")
                        h2f = (
                            h2_sb if io_dt == mybir.dt.bfloat16
                            else h2_sb.bitcast(f32)
                        )
                        for d in range(KH):
                            nc.vector.tensor_scalar_mul(
                                hwf[:, d, :ct], h2f[:, d, :], w3_col(d),
                            )
                            if d:
                                nc.vector.tensor_add(
                                    hwf[:, 0, :ct], hwf[:, 0, :ct], hwf[:, d, :ct]
                                )
                        nc.scalar.activation(
                            hws[:, :ct], hwf[:, 0, :ct],
                            mybir.ActivationFunctionType.Copy,
                        )
                        nc.tensor.matmul(
                            ps3[:], ones_sb[:], hws[:, :ct], start=True, stop=True
                        )
                    y_sb = outp.tile([1, maxct], f32, tag="y", name="y_sb")
                    nc.vector.tensor_copy(y_sb[:, :ct], ps3[:])
                    roff = sum(cts[:t])
                    nc.sync.dma_start(
                        y_d[e][0:1, roff : roff + ct], y_sb[:, :ct]
                    )
    return nc


def _tiles_for(mx):
    """Tile widths for one slot: first tile 256 (fast start), rest <=512,
    32-col aligned (64B at bf16)."""
    nt = max(1, -(-mx // 512))
    if nt == 1:
        return [max(256, ((mx + 31) // 32) * 32)]
    rest = -(-(mx - 256) // (nt - 1))
    rest = max(256, ((rest + 31) // 32) * 32)
    if rest > 512:
        nt += 1
        rest = max(256, ((-(-(mx - 256) // (nt - 1)) + 31) // 32) * 32)
    return [256] + [rest] * (nt - 1)


def _route(state, actions):
    """Sort rows by action; pick per-slot capacities at runtime.

    Slot 0 gets the 8 most-loaded experts, slot 1 the 8 least-loaded, so
    slot 1's shared capacity only covers the 9th-largest count."""
    order = np.argsort(actions, kind="stable")
    counts = np.bincount(actions, minlength=A)
    ranks = np.argsort(-counts, kind="stable")
    assign = [(int(ranks[i]), int(ranks[NCORES + i])) for i in range(NCORES)]
    ctss = [
        _tiles_for(max(1, int(counts[ranks[:NCORES]].max()))),
        _tiles_for(max(1, int(counts[ranks[NCORES:]].max()))),
    ]
    caps = [int(sum(cts)) for cts in ctss]
    return order, counts, assign, caps, ctss


def _build_inputs(state, W1, W2, W3, b1, b2, order, counts, assign, caps, ctss):
    _, np_dt = _dtypes()
    starts = np.zeros(A + 1, dtype=np.int64)
    starts[1:] = np.cumsum(counts)
    geom = [_xt_geom(cts) for cts in ctss]

    in_maps = []
    for core in range(NCORES):
        im = {}
        es = list(assign[core])
        for e in range(EPC):
            a = es[e]
            XW, boffs, dws = geom[e]
            cts = ctss[e]
            xt = np.zeros((P, XW), dtype=np_dt)
            idx = order[starts[a] : starts[a + 1]]
            n = len(idx)
            if n:
                # [P, KD, n]: feature d*128+p on partition p
                rows = (
                    state[idx].T.astype(np_dt)
                    .reshape(KD, P, n).transpose(1, 0, 2)
                )
                r0 = 0
                for t, ct in enumerate(cts):
                    r1 = min(n, r0 + ct)
                    if r1 <= r0:
                        break
                    blk = xt[:, boffs[t] : boffs[t] + KD * dws[t]].reshape(
                        P, KD, dws[t]
                    )
                    pad = BPAD if t == 0 else 0
                    blk[:, :, pad : pad + r1 - r0] = rows[:, :, r0:r1]
                    r0 = r1
            # head of tile-0 d=0 chunk: b1 | b2 | w3
            xt[:, 0:KH] = b1[a].astype(np_dt).reshape(KH, P).T
            xt[:, KH : 2 * KH] = b2[a].astype(np_dt).reshape(KH, P).T
            xt[:, 2 * KH : 3 * KH] = (
                W3[a][:, 0].astype(np_dt).reshape(KH, P).T
            )
            im[f"xt{e}"] = xt
        # h-major per partition: w[e, p, h, d, c] = W[a][d*128+p, h*128+c]
        im["w1"] = np.ascontiguousarray(
            W1[es].reshape(EPC, KD, P, KH, P)
            .transpose(0, 2, 3, 1, 4).astype(np_dt)
        )
        im["w2"] = np.ascontiguousarray(
            W2[es].reshape(EPC, KH, P, KH, P)
            .transpose(0, 2, 3, 1, 4).astype(np_dt)
        )
        in_maps.append(im)
    return in_maps


def _scatter(results, meta):
    order, counts, assign, b3, B = (
        meta["order"], meta["counts"], meta["assign"], meta["b3"], meta["B"]
    )
    starts = np.zeros(A + 1, dtype=np.int64)
    starts[1:] = np.cumsum(counts)
    out = np.empty((B, 1), dtype=np.float32)
    for core in range(NCORES):
        for e in range(EPC):
            a = assign[core][e]
            idx = order[starts[a] : starts[a + 1]]
            out[idx, 0] = results[core][f"y{e}"][0, : len(idx)] + b3[a, 0]
    return out


def run_spmd(nc, in_maps, **kw):
    from concourse.bass_utils import run_bass_kernel_spmd

    _install_bir_patch()
    return run_bass_kernel_spmd(nc, in_maps, core_ids=list(range(NCORES)), **kw)


def prepare(state, W1, b1, W2, b2, W3, b3, actions):
    state = np.asarray(state, dtype=np.float32)
    W1 = np.asarray(W1, dtype=np.float32)
    b1 = np.asarray(b1, dtype=np.float32)
    W2 = np.asarray(W2, dtype=np.float32)
    b2 = np.asarray(b2, dtype=np.float32)
    W3 = np.asarray(W3, dtype=np.float32)
    b3 = np.asarray(b3, dtype=np.float32)
    actions = np.asarray(actions).astype(np.int64)
    order, counts, assign, caps, ctss = _route(state, actions)
    nc = build_nc(caps, ctss)
    in_maps = _build_inputs(
        state, W1, W2, W3, b1, b2, order, counts, assign, caps, ctss
    )
    meta = {
        "order": order, "counts": counts, "assign": assign,
        "caps": caps, "b3": b3, "B": state.shape[0],
    }
    return nc, in_maps, meta


def kernel(state, W1, b1, W2, b2, W3, b3, actions):
    nc, in_maps, meta = prepare(state, W1, b1, W2, b2, W3, b3, actions)
    res = run_spmd(nc, in_maps)
    return _scatter(res.results, meta)


if __name__ == "__main__":
    rng = np.random.default_rng(0)
    B = 4096
    inputs = {
        "state": rng.standard_normal((B, D), dtype=np.float32),
        "W1": rng.standard_normal((A, D, H), dtype=np.float32) / np.sqrt(D),
        "b1": rng.standard_normal((A, H), dtype=np.float32) / np.sqrt(D),
        "W2": rng.standard_normal((A, H, H), dtype=np.float32) / np.sqrt(H),
        "b2": rng.standard_normal((A, H), dtype=np.float32) / np.sqrt(H),
        "W3": rng.standard_normal((A, H, 1), dtype=np.float32) / np.sqrt(H),
        "b3": rng.standard_normal((A, 1), dtype=np.float32) / np.sqrt(H),
        "actions": rng.integers(0, A, B),
    }
    out = kernel(**inputs)
    h1 = np.maximum(
        np.einsum("bd,adh->bah", inputs["state"], inputs["W1"]) + inputs["b1"], 0
    )
    h2 = np.maximum(np.einsum("bah,ahk->bak", h1, inputs["W2"]) + inputs["b2"], 0)
    ref = np.einsum("bah,ahk->bak", h2, inputs["W3"]) + inputs["b3"]
    ref = np.take_along_axis(ref, inputs["actions"][:, None, None], axis=1)[:, 0, :]
    err = np.abs(out - ref).max() / np.abs(ref).max()
    print("self-check rel err:", err)


# revision 22
# speedup vs baseline: 1.0785x; 1.0785x over previous
"""Action-separated MLP (MoE routing) Trainium2 kernel.

Reference computes all 16 per-action MLPs for every row, then gathers the
selected action's output.  Only the selected expert's output survives, so we
route instead: sort rows by action on the host, run each expert's
512->512->512->1 MLP only on its own rows, and scatter back.  16x fewer FLOPs
than the dense reference.

Distribution: 16 experts over 8 cores, 2 experts per core.  Slot 0 holds the
8 most-loaded experts, slot 1 the 8 least-loaded, each slot padded to its own
capacity, so the shared SPMD NEFF pads to cap0+cap1 columns instead of
2*max_count.

Layout: activations are kept transposed (features on SBUF partitions, rows on
the free dim), which makes every layer a plain lhsT.T @ rhs chain with the
per-partition bias + ReLU fused into one ScalarEngine activation op.  The
DRAM image of each expert's xt is tile-contiguous per partition (a tile's 4
contraction chunks adjacent), so each DMA moves 128 x 2.3-3.5KB contiguous
runs -- descriptor-efficient.  b1/b2 biases and the tiny W3 column ride in
BPAD spare columns at the head of the tile-0 block (dedicated small DMAs are
descriptor-overhead-bound).

Matmuls run in bf16 (inputs quantized on host): 1 cycle/row streaming and
LDWEIGHTS gets the compiler-automatic Fast Weight Load (disabled for fp32
dtypes), so weight loads hide behind the matmul stream.  Measured rel err vs
the fp32 reference ~5e-3, inside the 2e-2 gate.

Perf notes (from NTFF traces): the PE is stream-bound; the measured exec
window starts at the framework's const-AP memsets (~6.3us in), so the
preamble before that is free, but walrus's end-of-NEFF re-execution epilogue
(one semaphore clear per sem, split across engines) is fully paid.  The BIR
patch compacts semaphore IDs and caps --max-sem-num to shrink it.
"""

import json
import sys

import numpy as np

sys.path.insert(0, "/opt/trn_rl_repo")

import ml_dtypes  # noqa: E402

import concourse.bass as bass  # noqa: E402
import concourse.mybir as mybir  # noqa: E402
import concourse.tile as tile  # noqa: E402
from concourse.vector_clock import ScopedClock  # noqa: E402

A, D, H = 16, 512, 512
NCORES = 8
EPC = 2  # experts per core
P = 128
KD = D // P  # 4 contraction chunks for layer 1
KH = H // P  # 4 contraction chunks for layers 2/3
N_WARM = 32  # dummy matmuls bridging the gap until the first xt piece lands

# "bf16": cast inputs to bf16 on host, 1 cycle/row matmuls, least DMA, FWL.
# "f32r": fp32 storage, float32r matmuls (1 cycle/row at N>=256, no FWL).
DT_MODE = "bf16"

# spare columns at the head of the tile-0 block, per d-chunk; d=0's head
# carries b1 (cols 0:4), b2 (4:8) and the W3 column (8:12)
BPAD = 32 if DT_MODE == "bf16" else 16

# --max-sem-num handed to walrus for the current compile (set per-BIR by
# _patch_bir below; None = leave walrus at its default)
_MAX_SEM = [None]


def _patch_bir(ant_bir_str):
    """Two BIR rewrites before walrus:

    1. This walrus build rejects >1 embedded sync-wait per instruction.
       Move extra waits onto standalone EventSemaphore ops just before the
       owning instruction (same engine, so program order is preserved).
    2. Compact semaphore IDs to a dense range starting at 3, and cap
       --max-sem-num, to shrink walrus's end-of-NEFF per-semaphore clear
       epilogue (it is inside the measured exec window).
    """
    bir = json.loads(
        ant_bir_str.decode() if isinstance(ant_bir_str, bytes) else ant_bir_str
    )
    used = set()
    for fn in bir.get("functions", []):
        for bb in fn.get("blocks", []):
            for inst in bb.get("instructions", []):
                si = inst.get("sync_info") or {}
                for k in ("on_update", "on_wait"):
                    for u in si.get(k) or []:
                        if isinstance(u, dict) and u.get("sync_type") == "semaphore":
                            used.add(u["id"])
    remap = {old: 3 + i for i, old in enumerate(sorted(used))}
    _MAX_SEM[0] = 3 + len(used) + 8  # slack for walrus-internal sems
    names = bir.get("ant_sem_names")
    if isinstance(names, dict):
        bir["ant_sem_names"] = {
            str(remap.get(int(k), int(k))): v for k, v in names.items()
        }

    for fn in bir.get("functions", []):
        for bb in fn.get("blocks", []):
            new_insts = []
            for inst in bb.get("instructions", []):
                si = inst.get("sync_info") or {}
                for k in ("on_update", "on_wait"):
                    for u in si.get(k) or []:
                        if isinstance(u, dict) and u.get("sync_type") == "semaphore":
                            u["id"] = remap[u["id"]]
                waits = si.get("on_wait") or []
                if len(waits) > 1:
                    for j, w in enumerate(waits[:-1]):
                        new_insts.append(
                            {
                                "debug": inst.get("debug", 0),
                                "engine": inst["engine"],
                                "ins": [],
                                "name": f"{inst['name']}_xw{j}",
                                "opcode": "EventSemaphore",
                                "outs": [],
                                "sync_info": {"on_update": [], "on_wait": [w]},
                            }
                        )
                    si["on_wait"] = [waits[-1]]
                new_insts.append(inst)
            bb["instructions"] = new_insts
    return json.dumps(bir).encode()


def _install_bir_patch():
    from concourse import bass2jax, bass_utils

    if getattr(bass2jax.compile_bir_kernel, "_multiwait_patched", False):
        return

    orig = bass_utils.compile_bir_kernel

    def patched(ant_bir_str, tmpdir, neff_name="file.neff", **kw):
        return orig(_patch_bir(ant_bir_str), tmpdir, neff_name=neff_name, **kw)

    patched._multiwait_patched = True
    bass2jax.compile_bir_kernel = patched

    orig_gwa = bass_utils.get_walrus_args

    def gwa(*a, **kw):
        args = orig_gwa(*a, **kw)
        if _MAX_SEM[0] is not None:
            args = list(args) + [
                f"--max-sem-num={_MAX_SEM[0]}",
                "--trivial-semaphore-alloc",
            ]
        return args

    bass_utils.get_walrus_args = gwa


def _dtypes():
    if DT_MODE == "bf16":
        return mybir.dt.bfloat16, ml_dtypes.bfloat16
    if DT_MODE == "f32r":
        return mybir.dt.float32r, np.float32
    return mybir.dt.float32, np.float32


def _xt_geom(cts):
    """Flat per-expert xt layout: tile-0 block [KD, BPAD+ct0], then
    [KD, ct_t] blocks.  Returns (total_width, per-tile col base, per-tile
    d-chunk width)."""
    boffs, dws = [], []
    off = 0
    for t, ct in enumerate(cts):
        w = (BPAD + ct) if t == 0 else ct
        boffs.append(off)
        dws.append(w)
        off += KD * w
    return off, boffs, dws


def build_nc(caps, ctss):
    """Per-core Bass program: slot e handles capacity caps[e] = sum(ctss[e])."""
    io_dt, _ = _dtypes()
    f32 = mybir.dt.float32
    maxct = max(max(cts) for cts in ctss)
    geom = [_xt_geom(cts) for cts in ctss]

    nc = bass.Bass()
    xt_d = [
        nc.dram_tensor(f"xt{e}", [P, geom[e][0]], io_dt, kind="ExternalInput")
        for e in range(EPC)
    ]
    # weights are h-major per partition so the first consumed h-blocks are
    # contiguous leading pieces
    w1_d = nc.dram_tensor("w1", [EPC, P, KH, KD, P], io_dt, kind="ExternalInput")
    w2_d = nc.dram_tensor("w2", [EPC, P, KH, KH, P], io_dt, kind="ExternalInput")
    y_d = [
        nc.dram_tensor(f"y{e}", [1, caps[e]], f32, kind="ExternalOutput")
        for e in range(EPC)
    ]

    RELU = mybir.ActivationFunctionType.Relu

    def _light_drain(self, tick_clock, wait_clock):
        # Single-TileContext program: drain + one barrier is enough.  The
        # stock version adds a second all-engine barrier + semaphore clears
        # (for sem reuse by later contexts) costing ~1.3us of kernel tail;
        # re-execution of the same NEFF was verified safe without them.
        drain_inst = self.nc.sync.drain()
        wait_clock.add_sem_waits(
            drain_inst.ins, ScopedClock({None: tick_clock.global_clock})
        )
        self.nc.all_engine_barrier()
        popped = self.nc._tile_sem_poison_stack.pop()
        assert popped is self._sem_poison

    with tile.TileContext(nc) as tc:
        tc._drain_and_barrier = _light_drain.__get__(tc)
        with (
            tc.tile_pool(name="const", bufs=1) as const,
            tc.tile_pool(name="xt", bufs=2) as xtp,
            tc.tile_pool(name="wts", bufs=2) as wtp,
            tc.tile_pool(name="act", bufs=3) as actp,
            tc.tile_pool(name="out", bufs=4) as outp,
            tc.tile_pool(name="ps", bufs=5, space="PSUM") as psp,
            tc.tile_pool(name="ps3", bufs=2, space="PSUM") as ps3p,
            tc.tile_pool(name="pswarm", bufs=1, space="PSUM") as pswarmp,
        ):
            # PE warm-up: dummy matmuls bridge the PE from program start
            # until the first xt piece lands, keeping the HAM clock gate
            # (which watches PE *array* activity) un-throttled.
            warm_sb = const.tile([P, 64], io_dt, tag="warm")
            ones_sb = const.tile([P, 1], io_dt, tag="ones")
            if io_dt == mybir.dt.bfloat16:
                nc.vector.memset(warm_sb, 0.0)
                nc.vector.memset(ones_sb, 1.0)
            else:
                nc.vector.memset(warm_sb.bitcast(f32), 0.0)
                nc.vector.memset(ones_sb.bitcast(f32), 1.0)
            warm_ps = pswarmp.tile([64, 64], f32, tag="warm_ps")
            for _ in range(N_WARM):
                nc.tensor.matmul(warm_ps[:], warm_sb[:], warm_sb[:],
                                 start=True, stop=True)

            # Two trigger engines = two SW DMA queues, each spreading packets
            # over the 16 HW DMA engines; each trigger costs ~650ns on its
            # issuing engine, so few big tile-contiguous transfers beat many
            # small ones.  Pieces are ordered so each lands just before its
            # first consumer (queues complete in order).
            xt_sb = {}
            w_sb = {}
            for e in range(EPC):
                xt_sb[e] = xtp.tile(
                    [P, geom[e][0]], io_dt, tag=f"xt{e}", name=f"xt_sb{e}"
                )
                w1_sb = wtp.tile([P, KH, KD, P], io_dt, tag="w1", name=f"w1_sb{e}")
                w2_sb = wtp.tile([P, KH, KH, P], io_dt, tag="w2", name=f"w2_sb{e}")
                w_sb[e] = (w1_sb, w2_sb)

            def xt_piece(e, t, frac=(0, 1)):
                # piece frac[0] of frac[1] equal column ranges of tile t
                _, boffs, dws = geom[e]
                w = KD * dws[t]
                lo = boffs[t] + frac[0] * w // frac[1]
                hi = boffs[t] + (frac[0] + 1) * w // frac[1]
                return (xt_sb[e][:, lo:hi], xt_d[e][:, lo:hi])

            def w_piece(which, e, hlo, hhi):
                sb = w_sb[e][which]
                dr = (w1_d, w2_d)[which]
                return (sb[:, hlo:hhi, :, :], dr[e, :, hlo:hhi, :, :])

            # The DMA queues deliver only ~70GB/s each for the first few us,
            # so the pieces gating the first matmul groups are small and
            # spread across all three trigger queues, ordered by deadline.
            sched = [
                (nc.sync,   xt_piece(0, 0, (0, 2))),
                (nc.scalar, w_piece(0, 0, 0, 1)),
                (nc.gpsimd, xt_piece(0, 0, (1, 2))),
                (nc.sync,   w_piece(0, 0, 1, 2)),
                (nc.scalar, w_piece(0, 0, 2, 3)),
                (nc.gpsimd, w_piece(0, 0, 3, 4)),
                (nc.sync,   w_piece(1, 0, 0, 2)),
                (nc.scalar, w_piece(1, 0, 2, 4)),
                (nc.gpsimd, xt_piece(0, 1)),
                (nc.sync,   xt_piece(0, 2)),
                (nc.gpsimd, w_piece(0, 1, 0, 4)),
                (nc.sync,   w_piece(1, 1, 0, 4)),
                (nc.gpsimd, xt_piece(1, 1)),
                (nc.sync,   xt_piece(1, 2)),
                (nc.gpsimd, xt_piece(1, 0)),
            ]
            for eng, (dst, src) in sched:
                eng.dma_start(dst, src)

            for e in range(EPC):
                cts = ctss[e]
                _, boffs, dws = geom[e]
                nt = len(cts)
                w1_sb, w2_sb = w_sb[e]
                bias = xt_sb[e]
                if io_dt != mybir.dt.bfloat16:
                    bias = bias.bitcast(f32)
                    w3f = bias
                else:
                    # tensor_scalar wants a float32 scalar operand; make a
                    # tiny f32 copy of the packed W3 column once per expert
                    w3f = actp.tile([P, KH], f32, tag="w3f", name="w3f")
                    nc.scalar.activation(
                        w3f[:, :], xt_sb[e][:, 2 * KH : 3 * KH],
                        mybir.ActivationFunctionType.Copy,
                    )

                def w3_col(d):
                    if io_dt == mybir.dt.bfloat16:
                        return w3f[:, d : d + 1]
                    return bias[:, 2 * KH + d : 2 * KH + d + 1]

                def rhs(t, d, ct):
                    lo = boffs[t] + d * dws[t] + (BPAD if t == 0 else 0)
                    return xt_sb[e][:, lo : lo + ct]

                t_order = range(nt) if e == 0 else [*range(1, nt), 0]
                for t in t_order:
                    ct = cts[t]
                    h1_sb = actp.tile([P, KH, maxct], io_dt, tag="h1", name="h1_sb")[:, :, :ct]
                    for h in range(KH):
                        ps = psp.tile([P, maxct], f32, tag="ps", name="ps")[:, :ct]
                        for d in range(KD):
                            nc.tensor.matmul(
                                ps[:],
                                w1_sb[:, h, d, :],
                                rhs(t, d, ct),
                                start=(d == 0),
                                stop=(d == KD - 1),
                            )
                        nc.scalar.activation(
                            h1_sb[:, h, :], ps[:], RELU,
                            bias=bias[:, h : h + 1],
                        )
                    h2_sb = actp.tile([P, KH, maxct], io_dt, tag="h2", name="h2_sb")[:, :, :ct]
                    for h in range(KH):
                        ps = psp.tile([P, maxct], f32, tag="ps", name="ps")[:, :ct]
                        for d in range(KH):
                            nc.tensor.matmul(
                                ps[:],
                                w2_sb[:, h, d, :],
                                h1_sb[:, d, :],
                                start=(d == 0),
                                stop=(d == KH - 1),
                            )
                        nc.scalar.activation(
                            h2_sb[:, h, :], ps[:], RELU,
                            bias=bias[:, KH + h : KH + h + 1],
                        )
                    ps3 = ps3p.tile([1, maxct], f32, tag="ps3", name="ps3")[:, :ct]
                    if e == EPC - 1 and t == t_order[-1]:
                        # final tile: classic 4-matmul reduction -- the PE is
                        # idle here and the DVE chain would be exposed tail
                        for d in range(KH):
                            nc.tensor.matmul(
                                ps3[:],
                                xt_sb[e][:, 2 * KH + d : 2 * KH + d + 1],
                                h2_sb[:, d, :],
                                start=(d == 0),
                                stop=(d == KH - 1),
                            )
                    else:
                        # bulk tiles: h2*w3 per chunk + chunk sum on the DVE,
                        # then ONE ones-vector matmul does the 128-partition
                        # reduction -- 1 PE row-stream instead of 4.  The sum
                        # bounces through ACT into a dedicated tensor (for
                        # f32r the BIR verifier accepts only
                        # Activation/DMA/Memset writers for matmul inputs;
                        # for bf16 it also casts f32 -> bf16).
                        hwf = actp.tile([P, KH, maxct], f32, tag="hwf", name="hwf")
                        hws = actp.tile([P, maxct], io_dt, tag="hws", name="hws")
                        h2f = (
                            h2_sb if io_dt == mybir.dt.bfloat16
                            else h2_sb.bitcast(f32)
                        )
                        for d in range(KH):
                            nc.vector.tensor_scalar_mul(
                                hwf[:, d, :ct], h2f[:, d, :], w3_col(d),
                            )
                            if d:
                                nc.vector.tensor_add(
                                    hwf[:, 0, :ct], hwf[:, 0, :ct], hwf[:, d, :ct]
                                )
                        nc.scalar.activation(
                            hws[:, :ct], hwf[:, 0, :ct],
                            mybir.ActivationFunctionType.Copy,
                        )
                        nc.tensor.matmul(
                            ps3[:], ones_sb[:], hws[:, :ct], start=True, stop=True
                        )
                    y_sb = outp.tile([1, maxct], f32, tag="y", name="y_sb")
                    nc.vector.tensor_copy(y_sb[:, :ct], ps3[:])
                    roff = sum(cts[:t])
                    nc.sync.dma_start(
                        y_d[e][0:1, roff : roff + ct], y_sb[:, :ct]
                    )
    return nc


def _tiles_for(mx):
    """Tile widths for one slot: first tile 256 (fast start), rest <=512,
    32-col aligned (64B at bf16)."""
    nt = max(1, -(-mx // 512))
    if nt == 1:
        return [max(256, ((mx + 31) // 32) * 32)]
    rest = -(-(mx - 256) // (nt - 1))
    rest = max(256, ((rest + 31) // 32) * 32)
    if rest > 512:
        nt += 1
        rest = max(256, ((-(-(mx - 256) // (nt - 1)) + 31) // 32) * 32)
    return [256] + [rest] * (nt - 1)


def _route(state, actions):
    """Sort rows by action; pick per-slot capacities at runtime.

    Slot 0 gets the 8 most-loaded experts, slot 1 the 8 least-loaded, so
    slot 1's shared capacity only covers the 9th-largest count."""
    order = np.argsort(actions, kind="stable")
    counts = np.bincount(actions, minlength=A)
    ranks = np.argsort(-counts, kind="stable")
    assign = [(int(ranks[i]), int(ranks[NCORES + i])) for i in range(NCORES)]
    ctss = [
        _tiles_for(max(1, int(counts[ranks[:NCORES]].max()))),
        _tiles_for(max(1, int(counts[ranks[NCORES:]].max()))),
    ]
    caps = [int(sum(cts)) for cts in ctss]
    return order, counts, assign, caps, ctss


def _build_inputs(state, W1, W2, W3, b1, b2, order, counts, assign, caps, ctss):
    _, np_dt = _dtypes()
    starts = np.zeros(A + 1, dtype=np.int64)
    starts[1:] = np.cumsum(counts)
    geom = [_xt_geom(cts) for cts in ctss]

    in_maps = []
    for core in range(NCORES):
        im = {}
        es = list(assign[core])
        for e in range(EPC):
            a = es[e]
            XW, boffs, dws = geom[e]
            cts = ctss[e]
            xt = np.zeros((P, XW), dtype=np_dt)
            idx = order[starts[a] : starts[a + 1]]
            n = len(idx)
            if n:
                # [P, KD, n]: feature d*128+p on partition p
                rows = (
                    state[idx].T.astype(np_dt)
                    .reshape(KD, P, n).transpose(1, 0, 2)
                )
                r0 = 0
                for t, ct in enumerate(cts):
                    r1 = min(n, r0 + ct)
                    if r1 <= r0:
                        break
                    blk = xt[:, boffs[t] : boffs[t] + KD * dws[t]].reshape(
                        P, KD, dws[t]
                    )
                    pad = BPAD if t == 0 else 0
                    blk[:, :, pad : pad + r1 - r0] = rows[:, :, r0:r1]
                    r0 = r1
            # head of tile-0 d=0 chunk: b1 | b2 | w3
            xt[:, 0:KH] = b1[a].astype(np_dt).reshape(KH, P).T
            xt[:, KH : 2 * KH] = b2[a].astype(np_dt).reshape(KH, P).T
            xt[:, 2 * KH : 3 * KH] = (
                W3[a][:, 0].astype(np_dt).reshape(KH, P).T
            )
            im[f"xt{e}"] = xt
        # h-major per partition: w[e, p, h, d, c] = W[a][d*128+p, h*128+c]
        im["w1"] = np.ascontiguousarray(
            W1[es].reshape(EPC, KD, P, KH, P)
            .transpose(0, 2, 3, 1, 4).astype(np_dt)
        )
        im["w2"] = np.ascontiguousarray(
            W2[es].reshape(EPC, KH, P, KH, P)
            .transpose(0, 2, 3, 1, 4).astype(np_dt)
        )
        in_maps.append(im)
    return in_maps


def _scatter(results, meta):
    order, counts, assign, b3, B = (
        meta["order"], meta["counts"], meta["assign"], meta["b3"], meta["B"]
    )
    starts = np.zeros(A + 1, dtype=np.int64)
    starts[1:] = np.cumsum(counts)
    out = np.empty((B, 1), dtype=np.float32)
    for core in range(NCORES):
        for e in range(EPC):
            a = assign[core][e]
            idx = order[starts[a] : starts[a + 1]]
            out[idx, 0] = results[core][f"y{e}"][0, : len(idx)] + b3[a, 0]
    return out


def run_spmd(nc, in_maps, **kw):
    from concourse.bass_utils import run_bass_kernel_spmd

    _install_bir_patch()
    return run_bass_kernel_spmd(nc, in_maps, core_ids=list(range(NCORES)), **kw)


def prepare(state, W1, b1, W2, b2, W3, b3, actions):
    state = np.asarray(state, dtype=np.float32)
    W1 = np.asarray(W1, dtype=np.float32)
    b1 = np.asarray(b1, dtype=np.float32)
    W2 = np.asarray(W2, dtype=np.float32)
    b2 = np.asarray(b2, dtype=np.float32)
    W3 = np.asarray(W3, dtype=np.float32)
    b3 = np.asarray(b3, dtype=np.float32)
    actions = np.asarray(actions).astype(np.int64)
    order, counts, assign, caps, ctss = _route(state, actions)
    nc = build_nc(caps, ctss)
    in_maps = _build_inputs(
        state, W1, W2, W3, b1, b2, order, counts, assign, caps, ctss
    )
    meta = {
        "order": order, "counts": counts, "assign": assign,
        "caps": caps, "b3": b3, "B": state.shape[0],
    }
    return nc, in_maps, meta


def kernel(state, W1, b1, W2, b2, W3, b3, actions):
    nc, in_maps, meta = prepare(state, W1, b1, W2, b2, W3, b3, actions)
    res = run_spmd(nc, in_maps)
    return _scatter(res.results, meta)


if __name__ == "__main__":
    rng = np.random.default_rng(0)
    B = 4096
    inputs = {
        "state": rng.standard_normal((B, D), dtype=np.float32),
        "W1": rng.standard_normal((A, D, H), dtype=np.float32) / np.sqrt(D),
        "b1": rng.standard_normal((A, H), dtype=np.float32) / np.sqrt(D),
        "W2": rng.standard_normal((A, H, H), dtype=np.float32) / np.sqrt(H),
        "b2": rng.standard_normal((A, H), dtype=np.float32) / np.sqrt(H),
        "W3": rng.standard_normal((A, H, 1), dtype=np.float32) / np.sqrt(H),
        "b3": rng.standard_normal((A, 1), dtype=np.float32) / np.sqrt(H),
        "actions": rng.integers(0, A, B),
    }
    out = kernel(**inputs)
    h1 = np.maximum(
        np.einsum("bd,adh->bah", inputs["state"], inputs["W1"]) + inputs["b1"], 0
    )
    h2 = np.maximum(np.einsum("bah,ahk->bak", h1, inputs["W2"]) + inputs["b2"], 0)
    ref = np.einsum("bah,ahk->bak", h2, inputs["W3"]) + inputs["b3"]
    ref = np.take_along_axis(ref, inputs["actions"][:, None, None], axis=1)[:, 0, :]
    err = np.abs(out - ref).max() / np.abs(ref).max()
    print("self-check rel err:", err)
